# revision 14
# baseline (speedup 1.0000x reference)
"""Trainium2 Bass kernel for nn_DocREModel (doc-level relation extraction graph pooling).

Strategy (8 NeuronCores): each doc b (B=4) is split across 2 cores by attention
heads (6 heads each).  Every use of the attention tensor in the model is linear
in attention up to cheap scalar normalizations, so each core:
  - streams its [6,1024,1024] attention slice once from HBM,
  - accumulates the head-sum S[L,L] in SBUF (first head DMA'd straight into the
    accumulator, remaining heads added on the vector engine),
  - computes, via PE matmuls against host-built gather/mask matrices:
      GT     = S^T @ [onehotT|maskT]  (mention rows of S + span-row sums, both
                                       in contraction-major layout)
      v      = (uT*maskT)^T @ [seq|1]          (link-span numerator)
      mnum   = mrowsT^T @ [seq|1]              (mention-context numerator + row-sum)
      memb   = onehotT^T @ [seq|1]             (mention embeddings)
The host adds the two per-doc partials and applies the tiny normalizations
(head-count / span-length / row-sum divides, entity pooling, 4-way logsumexp)
while unsharding.
"""

import os
import sys

for _p in ("/opt/trn_rl_repo", "/root/.axon_site/_ro/trn_rl_repo"):
    if os.path.isdir(_p) and _p not in sys.path:
        sys.path.insert(0, _p)

import numpy as np

B, L, H, NH = 4, 1024, 768, 12
E, MPE, K = 32, 4, 16
EM = E * MPE              # 128 mentions per doc
TYPE_DIM = 20
OFFSET = 1
HPG = NH // 2             # heads per core (2 cores per doc)
RC = L // 128              # 8 chunks of 128 along L
HA = H + 2                # hidden + ones column (row-sum) + zero pad (fp32r needs even N)
RW = EM + K               # 144 columns of the combined gather/mask matrix

F32R_BIG = True           # float32r for the N>=256 contraction matmuls
F32R_GT = True            # float32r for the GT (S^T @ rmat) matmuls


def _build_nc(debug=False, f32r_big=F32R_BIG, f32r_gt=F32R_GT):
    import concourse.bass as bass
    import concourse.mybir as mybir
    import concourse.tile as tile
    from concourse import bacc

    f32 = mybir.dt.float32
    f32r = mybir.dt.float32r
    ts, ds = bass.ts, bass.ds

    dm = f32r if (f32r_big or f32r_gt) else f32   # dtype for matmul operands

    def big(ap):
        return ap

    def gtc(ap):
        return ap

    nc = bacc.Bacc("TRN2", target_bir_lowering=False, debug=debug)

    att6 = nc.dram_tensor("att6", [HPG * L, L], dm, kind="ExternalInput")
    seq_aug = nc.dram_tensor("seq_aug", [L, HA], dm, kind="ExternalInput")
    rmat = nc.dram_tensor("rmat", [L, RW], dm, kind="ExternalInput")
    out_v = nc.dram_tensor("out_v", [K, HA], f32, kind="ExternalOutput")
    out_mnum = nc.dram_tensor("out_mnum", [EM, HA], f32, kind="ExternalOutput")
    out_memb = nc.dram_tensor("out_memb", [EM, HA], f32, kind="ExternalOutput")

    with tile.TileContext(nc) as tc:
        with (
            tc.tile_pool(name="const", bufs=1) as constp,
            tc.tile_pool(name="stream", bufs=8) as streamp,
            tc.tile_pool(name="accum", bufs=1) as accp,
            tc.tile_pool(name="stage", bufs=1) as stagep,
            tc.tile_pool(name="psall", bufs=8, space="PSUM") as psall,
        ):
            # ---- constants: seq/rmat chunked along L (partition = position) ----
            seq_s = [constp.tile([128, HA], dm, tag=f"seq{rc}", name=f"seq{rc}") for rc in range(RC)]
            rmat_s = [constp.tile([128, RW], dm, tag=f"rmat{rc}", name=f"rmat{rc}") for rc in range(RC)]
            for rc in range(RC):
                nc.sync.dma_start(out=seq_s[rc][:], in_=seq_aug[ts(rc, 128), :])
                nc.scalar.dma_start(out=rmat_s[rc][:], in_=rmat[ts(rc, 128), :])

            # ---- mention embeddings memb = onehot^T @ [seq|1] (needs only consts) ----
            pmemb0 = psall.tile([EM, 512], f32, tag="ps", name="pmemb0")
            pmemb1 = psall.tile([EM, HA - 512], f32, tag="ps", name="pmemb1")
            for rc in range(RC):
                nc.tensor.matmul(pmemb0[:], big(rmat_s[rc][:, 0:EM]), big(seq_s[rc][:, 0:512]),
                                 start=(rc == 0), stop=(rc == RC - 1))
                nc.tensor.matmul(pmemb1[:], big(rmat_s[rc][:, 0:EM]), big(seq_s[rc][:, 512:HA]),
                                 start=(rc == 0), stop=(rc == RC - 1))
            memb_s = stagep.tile([EM, HA], f32, tag="memb", name="memb")
            nc.vector.tensor_copy(memb_s[:, 0:512], pmemb0[:])
            nc.vector.tensor_copy(memb_s[:, 512:HA], pmemb1[:])
            nc.sync.dma_start(out=out_memb[:], in_=memb_s[:])

            # ---- stream attention: h=0 lands straight in S, h>=1 in 1MB pair
            #      tiles (two row-chunks per transfer), added on DVE ----
            S_tiles = [accp.tile([128, L], dm, tag=f"S{rc}", name=f"S{rc}") for rc in range(RC)]
            gt_s = [accp.tile([128, RW], dm, tag=f"gt{ct}", name=f"gt{ct}") for ct in range(RC)]
            gt_ps = [psall.tile([128, RW], f32, tag="ps", name=f"gtp{ct}") for ct in range(RC)]
            att6_r = att6[:].rearrange("(h rcp p) c -> h rcp p c", h=HPG, p=128)

            for rc in range(RC):
                nc.sync.dma_start(out=S_tiles[rc][:], in_=att6[ds(rc * 128, 128), :])
            n_pairs = RC // 2
            for rp in range(n_pairs):
                for h in range(1, HPG):
                    t = streamp.tile([128, 2, L], dm, tag="att", name="att")
                    eng = nc.sync if (h % 2) else nc.scalar
                    eng.dma_start(out=t[:], in_=att6_r[h, ds(2 * rp, 2)].rearrange("rcp p c -> p rcp c"))
                    veng = nc.gpsimd if h == 1 else nc.vector
                    veng.tensor_add(S_tiles[2 * rp][:], S_tiles[2 * rp][:], t[:, 0, :])
                    veng.tensor_add(S_tiles[2 * rp + 1][:], S_tiles[2 * rp + 1][:], t[:, 1, :])
                # GT contributions once both chunks of the pair are complete;
                # each ct accumulates across all rc in its own PSUM bank
                for rc in (2 * rp, 2 * rp + 1):
                    for ct in range(RC):
                        nc.tensor.matmul(gt_ps[ct][:], gtc(S_tiles[rc][:, ts(ct, 128)]), gtc(rmat_s[rc][:]),
                                         start=(rc == 0), stop=(rc == RC - 1))
            for ct in range(RC):
                nc.vector.tensor_copy(gt_s[ct][:], gt_ps[ct][:])

            # ---- wvT = uT * maskT ----
            wv_s = [accp.tile([128, K], dm, tag=f"wv{ct}", name=f"wv{ct}") for ct in range(RC)]
            for ct in range(RC):
                nc.vector.tensor_mul(wv_s[ct][:], gt_s[ct][:, EM:RW], rmat_s[ct][:, EM:RW])

            # ---- contraction over positions: numerators for contexts + link reps ----
            pmnum0 = psall.tile([EM, 512], f32, tag="ps", name="pmnum0")
            pmnum1 = psall.tile([EM, HA - 512], f32, tag="ps", name="pmnum1")
            pv0 = psall.tile([K, 512], f32, tag="ps", name="pv0")
            pv1 = psall.tile([K, HA - 512], f32, tag="ps", name="pv1")
            for ct in range(RC):
                nc.tensor.matmul(pmnum0[:], big(gt_s[ct][:, 0:EM]), big(seq_s[ct][:, 0:512]),
                                 start=(ct == 0), stop=(ct == RC - 1))
                nc.tensor.matmul(pmnum1[:], big(gt_s[ct][:, 0:EM]), big(seq_s[ct][:, 512:HA]),
                                 start=(ct == 0), stop=(ct == RC - 1))
                nc.tensor.matmul(pv0[:], big(wv_s[ct][:]), big(seq_s[ct][:, 0:512]),
                                 start=(ct == 0), stop=(ct == RC - 1))
                nc.tensor.matmul(pv1[:], big(wv_s[ct][:]), big(seq_s[ct][:, 512:HA]),
                                 start=(ct == 0), stop=(ct == RC - 1))
            mnum_s = stagep.tile([EM, HA], f32, tag="mnum", name="mnum")
            nc.vector.tensor_copy(mnum_s[:, 0:512], pmnum0[:])
            nc.vector.tensor_copy(mnum_s[:, 512:HA], pmnum1[:])
            nc.sync.dma_start(out=out_mnum[:], in_=mnum_s[:])
            v_s = stagep.tile([K, HA], f32, tag="v", name="v")
            nc.vector.tensor_copy(v_s[:, 0:512], pv0[:])
            nc.vector.tensor_copy(v_s[:, 512:HA], pv1[:])
            nc.scalar.dma_start(out=out_v[:], in_=v_s[:])

    nc.compile()
    return nc


_NC_CACHE = {}


def _get_nc():
    if "nc" not in _NC_CACHE:
        _NC_CACHE["nc"] = _build_nc()
    return _NC_CACHE["nc"]


def _per_core_inputs(sequence_output, attention, mention_pos, link_start, link_len):
    """Returns (in_maps for 8 cores, per-doc span lengths)."""
    seq = np.ascontiguousarray(np.asarray(sequence_output, dtype=np.float32))
    att = np.asarray(attention)
    if att.dtype != np.float32:
        att = att.astype(np.float32)
    mpos = np.asarray(mention_pos).astype(np.int64)
    lstart = np.asarray(link_start).astype(np.int64)
    llen = np.asarray(link_len).astype(np.int64)

    in_maps = []
    lengths = []
    for b in range(B):
        pos = (mpos[b] + OFFSET).reshape(EM)
        onehotT = np.zeros((L, EM), np.float32)
        onehotT[pos, np.arange(EM)] = 1.0
        s = lstart[b] + OFFSET
        e = lstart[b] + llen[b] + 1 + OFFSET
        r = np.arange(L)
        maskT = ((r[:, None] >= s[None, :]) & (r[:, None] < e[None, :])).astype(np.float32)
        rmat = np.ascontiguousarray(np.concatenate([onehotT, maskT], axis=1))
        seq_aug = np.ascontiguousarray(
            np.concatenate([seq[b], np.ones((L, 1), np.float32), np.zeros((L, 1), np.float32)], axis=1))
        lengths.append((e - s).astype(np.float32))
        for g in range(2):
            att6 = np.ascontiguousarray(
                att[b, g * HPG:(g + 1) * HPG].reshape(HPG * L, L))
            in_maps.append({"att6": att6, "seq_aug": seq_aug, "rmat": rmat})
    return in_maps, lengths


def _combine(outs, lengths, type_table):
    ttab = np.asarray(type_table, dtype=np.float32)
    type_ids = np.concatenate(
        [np.zeros(E, np.int64), np.ones(EM, np.int64), np.full(K, 2, np.int64)])
    nodes_type = ttab[type_ids]  # [E+EM+K, TYPE_DIM]

    out = np.zeros((B, E + EM + K + E + EM, H + TYPE_DIM), np.float32)
    for b in range(B):
        o0, o1 = outs[2 * b], outs[2 * b + 1]
        v = o0["out_v"] + o1["out_v"]
        mnum = o0["out_mnum"] + o1["out_mnum"]
        memb = o0["out_memb"][:, :H]
        length = lengths[b]

        link_rep = v[:, :H] / (NH * length[:, None])
        m_ctx = mnum[:, :H] / (mnum[:, H:H + 1] + NH * 1e-5)
        enum = mnum.reshape(E, MPE, HA).sum(axis=1)
        e_ctx = enum[:, :H] / (enum[:, H:H + 1] + NH * MPE * 1e-5)

        mg = memb.reshape(E, MPE, H)
        mmax = mg.max(axis=1)
        eemb = np.log(np.exp(mg - mmax[:, None, :]).sum(axis=1)) + mmax

        nodes_raw = np.concatenate([eemb, memb, link_rep], axis=0)      # [176,H]
        nodes = np.concatenate([nodes_raw, nodes_type], axis=1)         # [176,H+20]
        ctx = np.concatenate([e_ctx, m_ctx], axis=0)                    # [160,H]
        ctx = np.concatenate([ctx, np.zeros((E + EM, TYPE_DIM), np.float32)], axis=1)
        out[b] = np.concatenate([nodes, ctx], axis=0)
    return out


def kernel(**inputs):
    from concourse.bass_utils import run_bass_kernel_spmd

    in_maps, lengths = _per_core_inputs(
        inputs["sequence_output"], inputs["attention"],
        inputs["mention_pos"], inputs["link_start"], inputs["link_len"])
    nc = _get_nc()
    res = run_bass_kernel_spmd(nc, in_maps, core_ids=list(range(8)))
    return _combine(res.results, lengths, inputs["type_table"])


# revision 16
# speedup vs baseline: 1.1039x; 1.1039x over previous
"""Trainium2 Bass kernel for nn_DocREModel (doc-level relation extraction graph pooling).

Strategy (8 NeuronCores): each doc b (B=4) is split across 2 cores by attention
heads (6 heads each).  Every use of the attention tensor in the model is linear
in attention up to cheap scalar normalizations, so each core:
  - streams its [6,1024,1024] attention slice once from HBM,
  - accumulates the head-sum S[L,L] in SBUF (first head DMA'd straight into the
    accumulator, remaining heads added on the vector engine),
  - computes, via PE matmuls against host-built gather/mask matrices:
      GT     = S^T @ [onehotT|maskT]  (mention rows of S + span-row sums, both
                                       in contraction-major layout)
      v      = (uT*maskT)^T @ [seq|1]          (link-span numerator)
      mnum   = mrowsT^T @ [seq|1]              (mention-context numerator + row-sum)
      memb   = onehotT^T @ [seq|1]             (mention embeddings)
The host adds the two per-doc partials and applies the tiny normalizations
(head-count / span-length / row-sum divides, entity pooling, 4-way logsumexp)
while unsharding.
"""

import os
import sys

for _p in ("/opt/trn_rl_repo", "/root/.axon_site/_ro/trn_rl_repo"):
    if os.path.isdir(_p) and _p not in sys.path:
        sys.path.insert(0, _p)

import numpy as np

B, L, H, NH = 4, 1024, 768, 12
E, MPE, K = 32, 4, 16
EM = E * MPE              # 128 mentions per doc
TYPE_DIM = 20
OFFSET = 1
HPG = NH // 2             # heads per core (2 cores per doc)
RC = L // 128              # 8 chunks of 128 along L
HA = H + 2                # hidden + ones column (row-sum) + zero pad (fp32r needs even N)
RW = EM + K               # 144 columns of the combined gather/mask matrix

F32R_BIG = True           # float32r for the N>=256 contraction matmuls
F32R_GT = True            # float32r for the GT (S^T @ rmat) matmuls


def _build_nc(debug=False, f32r_big=F32R_BIG, f32r_gt=F32R_GT):
    import concourse.bass as bass
    import concourse.mybir as mybir
    import concourse.tile as tile
    from concourse import bacc

    f32 = mybir.dt.float32
    f32r = mybir.dt.float32r
    ts, ds = bass.ts, bass.ds

    dm = f32r if (f32r_big or f32r_gt) else f32   # dtype for matmul operands

    def big(ap):
        return ap

    def gtc(ap):
        return ap

    nc = bacc.Bacc("TRN2", target_bir_lowering=False, debug=debug)

    att6 = nc.dram_tensor("att6", [HPG * L, L], dm, kind="ExternalInput")
    seq_aug = nc.dram_tensor("seq_aug", [L, HA], dm, kind="ExternalInput")
    rmat = nc.dram_tensor("rmat", [L, RW], dm, kind="ExternalInput")
    out_v = nc.dram_tensor("out_v", [K, HA], f32, kind="ExternalOutput")
    out_mnum = nc.dram_tensor("out_mnum", [EM, HA], f32, kind="ExternalOutput")
    out_memb = nc.dram_tensor("out_memb", [EM, HA], f32, kind="ExternalOutput")

    with tile.TileContext(nc) as tc:
        with (
            tc.tile_pool(name="const", bufs=1) as constp,
            tc.tile_pool(name="stream", bufs=8) as streamp,
            tc.tile_pool(name="accum", bufs=1) as accp,
            tc.tile_pool(name="stage", bufs=1) as stagep,
            tc.tile_pool(name="psall", bufs=8, space="PSUM") as psall,
        ):
            # ---- attention stream starts first (h=0 straight into S); consts
            #      interleave behind it so the HBM stream isn't delayed ----
            S_tiles = [accp.tile([128, L], dm, tag=f"S{rc}", name=f"S{rc}") for rc in range(RC)]
            for rc in range(RC):
                nc.sync.dma_start(out=S_tiles[rc][:], in_=att6[ds(rc * 128, 128), :])

            seq_s = [constp.tile([128, HA], dm, tag=f"seq{rc}", name=f"seq{rc}") for rc in range(RC)]
            rmat_s = [constp.tile([128, RW], dm, tag=f"rmat{rc}", name=f"rmat{rc}") for rc in range(RC)]
            for rc in range(RC):
                nc.scalar.dma_start(out=seq_s[rc][:], in_=seq_aug[ts(rc, 128), :])
                nc.scalar.dma_start(out=rmat_s[rc][:], in_=rmat[ts(rc, 128), :])

            # ---- mention embeddings memb = onehot^T @ [seq|1] (needs only consts) ----
            pmemb0 = psall.tile([EM, 512], f32, tag="ps", name="pmemb0")
            pmemb1 = psall.tile([EM, HA - 512], f32, tag="ps", name="pmemb1")
            for rc in range(RC):
                nc.tensor.matmul(pmemb0[:], big(rmat_s[rc][:, 0:EM]), big(seq_s[rc][:, 0:512]),
                                 start=(rc == 0), stop=(rc == RC - 1))
                nc.tensor.matmul(pmemb1[:], big(rmat_s[rc][:, 0:EM]), big(seq_s[rc][:, 512:HA]),
                                 start=(rc == 0), stop=(rc == RC - 1))
            memb_s = stagep.tile([EM, HA], f32, tag="memb", name="memb")
            nc.vector.tensor_copy(memb_s[:, 0:512], pmemb0[:])
            nc.vector.tensor_copy(memb_s[:, 512:HA], pmemb1[:])
            nc.sync.dma_start(out=out_memb[:], in_=memb_s[:])

            # ---- stream attention: h>=1 in 1MB pair tiles (two row-chunks per
            #      transfer), added on DVE; GT contributions per pair go through
            #      short-lived 2-matmul PSUM groups ----
            gt_s = [accp.tile([128, RW], dm, tag=f"gt{ct}", name=f"gt{ct}") for ct in range(RC)]
            att6_r = att6[:].rearrange("(h rcp p) c -> h rcp p c", h=HPG, p=128)

            n_pairs = RC // 2
            for rp in range(n_pairs):
                for h in range(1, HPG):
                    t = streamp.tile([128, 2, L], dm, tag="att", name="att")
                    eng = nc.sync if (h % 2) else nc.scalar
                    eng.dma_start(out=t[:], in_=att6_r[h, ds(2 * rp, 2)].rearrange("rcp p c -> p rcp c"))
                    nc.vector.tensor_add(S_tiles[2 * rp][:], S_tiles[2 * rp][:], t[:, 0, :])
                    nc.vector.tensor_add(S_tiles[2 * rp + 1][:], S_tiles[2 * rp + 1][:], t[:, 1, :])
                # GT contributions once both chunks of the pair are complete
                for ct in range(RC):
                    p = psall.tile([128, RW], f32, tag="ps", name="gtp")
                    nc.tensor.matmul(p[:], gtc(S_tiles[2 * rp][:, ts(ct, 128)]), gtc(rmat_s[2 * rp][:]),
                                     start=True, stop=False)
                    nc.tensor.matmul(p[:], gtc(S_tiles[2 * rp + 1][:, ts(ct, 128)]), gtc(rmat_s[2 * rp + 1][:]),
                                     start=False, stop=True)
                    if rp == 0:
                        nc.vector.tensor_copy(gt_s[ct][:], p[:])
                    else:
                        nc.vector.tensor_add(gt_s[ct][:], gt_s[ct][:], p[:])

            # ---- wvT = uT * maskT ----
            wv_s = [accp.tile([128, K], dm, tag=f"wv{ct}", name=f"wv{ct}") for ct in range(RC)]
            for ct in range(RC):
                nc.vector.tensor_mul(wv_s[ct][:], gt_s[ct][:, EM:RW], rmat_s[ct][:, EM:RW])

            # ---- contraction over positions: numerators for contexts + link reps ----
            pmnum0 = psall.tile([EM, 512], f32, tag="ps", name="pmnum0")
            pmnum1 = psall.tile([EM, HA - 512], f32, tag="ps", name="pmnum1")
            pv0 = psall.tile([K, 512], f32, tag="ps", name="pv0")
            pv1 = psall.tile([K, HA - 512], f32, tag="ps", name="pv1")
            for ct in range(RC):
                nc.tensor.matmul(pmnum0[:], big(gt_s[ct][:, 0:EM]), big(seq_s[ct][:, 0:512]),
                                 start=(ct == 0), stop=(ct == RC - 1))
                nc.tensor.matmul(pmnum1[:], big(gt_s[ct][:, 0:EM]), big(seq_s[ct][:, 512:HA]),
                                 start=(ct == 0), stop=(ct == RC - 1))
                nc.tensor.matmul(pv0[:], big(wv_s[ct][:]), big(seq_s[ct][:, 0:512]),
                                 start=(ct == 0), stop=(ct == RC - 1))
                nc.tensor.matmul(pv1[:], big(wv_s[ct][:]), big(seq_s[ct][:, 512:HA]),
                                 start=(ct == 0), stop=(ct == RC - 1))
            mnum_s = stagep.tile([EM, HA], f32, tag="mnum", name="mnum")
            nc.vector.tensor_copy(mnum_s[:, 0:512], pmnum0[:])
            nc.vector.tensor_copy(mnum_s[:, 512:HA], pmnum1[:])
            nc.sync.dma_start(out=out_mnum[:], in_=mnum_s[:])
            v_s = stagep.tile([K, HA], f32, tag="v", name="v")
            nc.vector.tensor_copy(v_s[:, 0:512], pv0[:])
            nc.vector.tensor_copy(v_s[:, 512:HA], pv1[:])
            nc.scalar.dma_start(out=out_v[:], in_=v_s[:])

    nc.compile()
    return nc


_NC_CACHE = {}


def _get_nc():
    if "nc" not in _NC_CACHE:
        _NC_CACHE["nc"] = _build_nc()
    return _NC_CACHE["nc"]


def _per_core_inputs(sequence_output, attention, mention_pos, link_start, link_len):
    """Returns (in_maps for 8 cores, per-doc span lengths)."""
    seq = np.ascontiguousarray(np.asarray(sequence_output, dtype=np.float32))
    att = np.asarray(attention)
    if att.dtype != np.float32:
        att = att.astype(np.float32)
    mpos = np.asarray(mention_pos).astype(np.int64)
    lstart = np.asarray(link_start).astype(np.int64)
    llen = np.asarray(link_len).astype(np.int64)

    in_maps = []
    lengths = []
    for b in range(B):
        pos = (mpos[b] + OFFSET).reshape(EM)
        onehotT = np.zeros((L, EM), np.float32)
        onehotT[pos, np.arange(EM)] = 1.0
        s = lstart[b] + OFFSET
        e = lstart[b] + llen[b] + 1 + OFFSET
        r = np.arange(L)
        maskT = ((r[:, None] >= s[None, :]) & (r[:, None] < e[None, :])).astype(np.float32)
        rmat = np.ascontiguousarray(np.concatenate([onehotT, maskT], axis=1))
        seq_aug = np.ascontiguousarray(
            np.concatenate([seq[b], np.ones((L, 1), np.float32), np.zeros((L, 1), np.float32)], axis=1))
        lengths.append((e - s).astype(np.float32))
        for g in range(2):
            att6 = np.ascontiguousarray(
                att[b, g * HPG:(g + 1) * HPG].reshape(HPG * L, L))
            in_maps.append({"att6": att6, "seq_aug": seq_aug, "rmat": rmat})
    return in_maps, lengths


def _combine(outs, lengths, type_table):
    ttab = np.asarray(type_table, dtype=np.float32)
    type_ids = np.concatenate(
        [np.zeros(E, np.int64), np.ones(EM, np.int64), np.full(K, 2, np.int64)])
    nodes_type = ttab[type_ids]  # [E+EM+K, TYPE_DIM]

    out = np.zeros((B, E + EM + K + E + EM, H + TYPE_DIM), np.float32)
    for b in range(B):
        o0, o1 = outs[2 * b], outs[2 * b + 1]
        v = o0["out_v"] + o1["out_v"]
        mnum = o0["out_mnum"] + o1["out_mnum"]
        memb = o0["out_memb"][:, :H]
        length = lengths[b]

        link_rep = v[:, :H] / (NH * length[:, None])
        m_ctx = mnum[:, :H] / (mnum[:, H:H + 1] + NH * 1e-5)
        enum = mnum.reshape(E, MPE, HA).sum(axis=1)
        e_ctx = enum[:, :H] / (enum[:, H:H + 1] + NH * MPE * 1e-5)

        mg = memb.reshape(E, MPE, H)
        mmax = mg.max(axis=1)
        eemb = np.log(np.exp(mg - mmax[:, None, :]).sum(axis=1)) + mmax

        nodes_raw = np.concatenate([eemb, memb, link_rep], axis=0)      # [176,H]
        nodes = np.concatenate([nodes_raw, nodes_type], axis=1)         # [176,H+20]
        ctx = np.concatenate([e_ctx, m_ctx], axis=0)                    # [160,H]
        ctx = np.concatenate([ctx, np.zeros((E + EM, TYPE_DIM), np.float32)], axis=1)
        out[b] = np.concatenate([nodes, ctx], axis=0)
    return out


def kernel(**inputs):
    from concourse.bass_utils import run_bass_kernel_spmd

    in_maps, lengths = _per_core_inputs(
        inputs["sequence_output"], inputs["attention"],
        inputs["mention_pos"], inputs["link_start"], inputs["link_len"])
    nc = _get_nc()
    res = run_bass_kernel_spmd(nc, in_maps, core_ids=list(range(8)))
    return _combine(res.results, lengths, inputs["type_table"])
